# revision 30
# baseline (speedup 1.0000x reference)
"""Trainium2 Bass kernel for nn_MoEElementFusion (moe_routing).

Strategy (8 NeuronCores, SPMD, two launches with host routing in between):
  Phase 1 (token-data-parallel): each core takes 1/8 of the 8192 (view,token)
  columns. Host precomputes prw = pw @ rw so h and r are both direct
  projections of x (no serial h->r dependency):
      [r | h] = x @ [prw | pw] + [pb@rw | pb]     (fp16 on the PE, psum fp32)
      d2 = |r|^2 - 2 r.keys^T + |keys|^2
  r tiles come first so the d2 chain (DVE squares -> ones-matmul row sum ->
  keys matmul) overlaps the h matmuls; on the last chunk d2 runs BEFORE the
  h tiles so the kernel tail is just the final h store. Input DMAs are
  split across both HWDGE queues (sync + scalar) in first-use order.

  Host: tokens whose 4th/5th logit gap is under REPAIR_MARGIN get their d2
  row recomputed exactly in fp32; logits = -sqrt(max(d2,0)), stable top-4,
  softmax gates in fp32. Experts with < HOST_EXPERT_MAX selected tokens are
  computed on host (128 weight tiles per slot put a ~19us LDWEIGHTS floor
  on any slot, which tiny experts cannot amortize). The rest are packed:

  Packing insight: every core executes the same slot-length profile
  (padding columns carry zero gates), so per-core balance is irrelevant -
  only the profile sum matters. Each expert is cut into n_e equalized
  expert-pure pieces (<= 512 cols, psum-bank limit); a small search over
  n_e minimizes sum_s max(chunk_s) + per-slot overhead, where chunks are
  consecutive groups of 8 pieces ordered expert-major. Giant experts fill
  whole chunks, and cores are aligned so consecutive same-expert slots
  REUSE the loaded weights (skip the 4MB weight DMA uniformly across
  cores, which kills the startup DMA crunch).

  Phase 2 (compiled at runtime once the profile is known): per slot s of
  length L_s, FFN in fp16 (1 cycle/row on the PE):
      out^T = (w2^T-mm(gelu(w1^T-mm(h^T) + b1)) + b2) * gates
  w2 accumulation is split into two psum groups (mo 0-1 interleaved one
  m-step behind gelu to hide ACT latency, mo 2-3 as a deferred second pass)
  so psum drains always overlap matmuls; slot 0 keeps its A pass w1-only so
  the PE never waits on the still-streaming w2. Weights triple-buffered;
  DMAs are emitted one slot ahead, w1+gates on the sync queue and h+w2 on
  the scalar queue, with output stores last so they never head-of-line
  block the next slot's input streams. psum tiles are allocated at a fixed
  512-column stride (bank alignment) and sliced to L_s.

  Host combine: fused[:, tok] += out columns per slot; sum the two views.
"""

import math
import os

import numpy as np

import concourse.bass as bass
import concourse.bacc as bacc
import concourse.mybir as mybir
import concourse.tile as tile
from concourse.bass_utils import run_bass_kernel_spmd

# Problem dims (hardcoded per spec)
V, B, T, D, E, K = 2, 4, 1024, 512, 16, 4
H = 4 * D
N = B * T          # tokens per view
NT = V * N         # total (view, token) columns = 8192
NC = 8             # cores
PC = NT // NC      # phase-1 columns per core = 1024
LMAX = 512         # max phase-2 slot length (psum bank limit)

F32 = mybir.dt.float32
F16 = mybir.dt.float16
AF = mybir.ActivationFunctionType
ALU = mybir.AluOpType

DK = D // 128      # 4 k-tiles over D
HK = H // 128      # 16 k-tiles over H

REPAIR_MARGIN = 0.02
HOST_EXPERT_MAX = 256   # experts with fewer selected tokens run on host

# Filled by kernel() for test harness introspection.
last_stats: dict = {}

_NC_CACHE: dict = {}


def _phase1_nc() -> bass.Bass:
    if "p1" in _NC_CACHE:
        return _NC_CACHE["p1"]
    nc = bacc.Bacc("TRN2", target_bir_lowering=False, num_devices=NC)
    xT = nc.dram_tensor("xT", [D, PC], F16, kind="ExternalInput")
    # wc = [prw | pw]: router projection first (d2 path starts earlier)
    wc = nc.dram_tensor("wc", [D, 2 * D], F16, kind="ExternalInput")
    pbc = nc.dram_tensor("pbc", [128, 2 * DK], F32, kind="ExternalInput")
    kT2 = nc.dram_tensor("kT2", [D, E], F16, kind="ExternalInput")
    konr = nc.dram_tensor("konr", [1, 512 + E], F32, kind="ExternalInput")
    onc = nc.dram_tensor("onc", [128, 1], F16, kind="ExternalInput")
    hT = nc.dram_tensor("hT", [D, PC], F16, kind="ExternalOutput")
    d2T = nc.dram_tensor("d2T", [E, PC], F32, kind="ExternalOutput")

    with tile.TileContext(nc) as tc:
        with (
            tc.tile_pool(name="const", bufs=1) as cpool,
            tc.tile_pool(name="act", bufs=1) as apool,
            tc.tile_pool(name="ps", bufs=4, space="PSUM") as pspool,
            tc.tile_pool(name="ps_small", bufs=2, space="PSUM") as psmall,
        ):
            xT_sb = cpool.tile([128, DK, PC], F16, tag="xT")
            wc_sb = cpool.tile([128, DK, 2 * D], F16, tag="wc")
            # Feed order = first-use order, split across both HWDGE queues:
            # prw slabs + first xT chunk fine-grained, the rest merged.
            for k in range(DK):
                q = nc.sync if k % 2 == 0 else nc.scalar
                q.dma_start(
                    wc_sb[:, k, 0:D], wc[k * 128 : (k + 1) * 128, 0:D]
                )
                q.dma_start(
                    xT_sb[:, k, 0:512], xT[k * 128 : (k + 1) * 128, 0:512]
                )
            nc.sync.dma_start(
                wc_sb[:, :, D : 2 * D],
                wc.rearrange("(k p) d -> p k d", p=128)[:, :, D : 2 * D],
            )
            nc.scalar.dma_start(
                xT_sb[:, :, 512:PC],
                xT.rearrange("(k p) n -> p k n", p=128)[:, :, 512:PC],
            )
            pbc_sb = cpool.tile([128, 2 * DK], F32, tag="pbc")
            nc.scalar.dma_start(pbc_sb[:], pbc[:])
            kT2_sb = cpool.tile([128, DK, E], F16, tag="kT2")
            nc.scalar.dma_start(
                kT2_sb[:], kT2.rearrange("(k p) e -> p k e", p=128)
            )
            konr_sb = cpool.tile([1, 512 + E], F32, tag="konr")
            nc.scalar.dma_start(konr_sb[:], konr[:])
            onc_sb = cpool.tile([128, 1], F16, tag="onc")
            nc.scalar.dma_start(onc_sb[:], onc[:])

            onr_sb = konr_sb[:, 0:512]
            kk_sb = konr_sb[:, 512 : 512 + E]

            hT_sb = apool.tile([128, DK, PC], F16, tag="hT")
            rT_sb = apool.tile([128, DK, PC], F16, tag="rT")
            r2_sb = apool.tile([128, DK, PC], F16, tag="r2")
            rr_sb = apool.tile([1, PC], F32, tag="rr")
            d2_sb = apool.tile([E, PC], F32, tag="d2")

            def big_tiles(nn, ms):
                cols = slice(nn * 512, (nn + 1) * 512)
                for m in ms:
                    ps = pspool.tile([128, 512], F32, tag="ps")
                    for k in range(DK):
                        nc.tensor.matmul(
                            ps[:],
                            wc_sb[:, k, m * 128 : (m + 1) * 128],
                            xT_sb[:, k, cols],
                            start=(k == 0),
                            stop=(k == DK - 1),
                        )
                    dst = rT_sb if m < DK else hT_sb
                    mm = m if m < DK else m - DK
                    nc.scalar.activation(
                        dst[:, mm, cols], ps[:],
                        AF.Identity, bias=pbc_sb[:, m : m + 1],
                    )
                    if m == DK - 1:
                        for kt in range(DK):
                            nc.vector.tensor_mul(
                                r2_sb[:, kt, cols], rT_sb[:, kt, cols],
                                rT_sb[:, kt, cols],
                            )

            def d2_chain(nn):
                cols = slice(nn * 512, (nn + 1) * 512)
                ps1 = psmall.tile([1, 512], F32, tag="ps1")
                for k in range(DK):
                    nc.tensor.matmul(
                        ps1[:], onc_sb[:], r2_sb[:, k, cols],
                        start=(k == 0), stop=(k == DK - 1),
                    )
                nc.scalar.activation(rr_sb[:, cols], ps1[:], AF.Copy)
                psA = psmall.tile([E, 512], F32, tag="psA")
                for k in range(DK):
                    nc.tensor.matmul(
                        psA[:], kT2_sb[:, k, :], rT_sb[:, k, cols],
                        start=(k == 0), stop=False,
                    )
                nc.tensor.matmul(psA[:], kk_sb, onr_sb, start=False, stop=False)
                nc.tensor.matmul(
                    psA[:], onr_sb[:, 0:E], rr_sb[:, cols],
                    start=False, stop=True,
                )
                nc.scalar.activation(d2_sb[:, cols], psA[:], AF.Copy)
                nc.scalar.dma_start(d2T[:, cols], d2_sb[:, cols])

            def h_out(nn):
                cols = slice(nn * 512, (nn + 1) * 512)
                nc.scalar.dma_start(
                    hT.rearrange("(m p) n -> p m n", p=128)[:, :, cols],
                    hT_sb[:, :, cols],
                )

            # chunk 0: r, h, then d2 (DVE overlaps the h matmuls);
            # chunk 1: r, d2, then h so the d2 chain hides under h matmuls
            # and the kernel tail is just the last h activation + store.
            big_tiles(0, range(2 * DK))
            h_out(0)
            d2_chain(0)
            big_tiles(1, range(DK))
            d2_chain(1)
            big_tiles(1, range(DK, 2 * DK))
            h_out(1)
    nc.compile()
    _NC_CACHE["p1"] = nc
    return nc


def _phase2_nc(Ls: tuple, reuse: tuple) -> bass.Bass:
    """FFN over S slots with compile-time lengths Ls (each <= 512)."""
    key = ("p2", Ls, reuse)
    if key in _NC_CACHE:
        return _NC_CACHE[key]
    nc = bacc.Bacc("TRN2", target_bir_lowering=False, num_devices=NC)
    S = len(Ls)
    offs = [0]
    for L in Ls:
        offs.append(offs[-1] + L)
    C = offs[-1]
    hseg = nc.dram_tensor("hseg", [D, C], F16, kind="ExternalInput")
    gseg = nc.dram_tensor("gseg", [128, C], F32, kind="ExternalInput")
    w1s = nc.dram_tensor("w1s", [S, D, H], F16, kind="ExternalInput")
    w2s = nc.dram_tensor("w2s", [S, H, D], F16, kind="ExternalInput")
    b1s = nc.dram_tensor("b1s", [128, S * HK], F32, kind="ExternalInput")
    b2s = nc.dram_tensor("b2s", [128, S * DK], F32, kind="ExternalInput")
    oseg = nc.dram_tensor("oseg", [D, C], F16, kind="ExternalOutput")

    with tile.TileContext(nc) as tc:
        with (
            tc.tile_pool(name="const", bufs=1) as cpool,
            tc.tile_pool(name="w1p", bufs=3) as w1p,
            tc.tile_pool(name="w2p", bufs=3) as w2p,
            tc.tile_pool(name="hp", bufs=3) as hp,
            tc.tile_pool(name="gp", bufs=3) as gp,
            tc.tile_pool(name="hidp", bufs=HK + 2) as hidp,
            tc.tile_pool(name="op", bufs=2) as op,
            tc.tile_pool(name="hid_ps", bufs=2, space="PSUM") as hidps,
            tc.tile_pool(name="outA_ps", bufs=1, space="PSUM") as outAps,
            tc.tile_pool(name="outB_ps", bufs=1, space="PSUM") as outBps,
        ):
            b1_sb = cpool.tile([128, S * HK], F32, tag="b1")
            b2_sb = cpool.tile([128, S * DK], F32, tag="b2")

            w1t = [None] * S
            w2t = [None] * S
            ht = [None] * S
            gt = [None] * S

            def emit_dmas(s):
                # w1 + gates on the sync HWDGE queue; h and w2 on the scalar
                # HWDGE queue (h first: A-pass needs it, w2 only at B-pass).
                # Slot 0 uses per-k descriptors so the first m-tiles arrive
                # early; prefetched slots use one descriptor per tensor to
                # cut issue time and semaphore teardown. Band slots with
                # reuse[s] keep the previous slot's weight tiles (no DMA).
                L = Ls[s]
                ht[s] = hp.tile([128, DK, L], F16, tag="h", name=f"ht_{s}")
                if s == 0:
                    for k in range(DK):
                        nc.scalar.dma_start(
                            ht[s][:, k, :],
                            hseg[k * 128 : (k + 1) * 128, offs[s] : offs[s] + L],
                        )
                else:
                    nc.scalar.dma_start(
                        ht[s][:],
                        hseg.rearrange("(k p) n -> p k n", p=128)[
                            :, :, offs[s] : offs[s] + L
                        ],
                    )
                if reuse[s]:
                    w1t[s] = w1t[s - 1]
                    w2t[s] = w2t[s - 1]
                else:
                    w1t[s] = w1p.tile(
                        [128, DK, H], F16, tag="w1", name=f"w1t_{s}"
                    )
                    if s == 0:
                        for k in range(DK):
                            for hh in range(2):
                                nc.sync.dma_start(
                                    w1t[s][
                                        :, k,
                                        hh * (H // 2) : (hh + 1) * (H // 2),
                                    ],
                                    w1s[
                                        s, k * 128 : (k + 1) * 128,
                                        hh * (H // 2) : (hh + 1) * (H // 2),
                                    ],
                                )
                    else:
                        nc.sync.dma_start(
                            w1t[s][:],
                            w1s[s].rearrange("(k p) h -> p k h", p=128),
                        )
                    w2t[s] = w2p.tile(
                        [128, HK, D], F16, tag="w2", name=f"w2t_{s}"
                    )
                    nc.scalar.dma_start(
                        w2t[s][:],
                        w2s[s].rearrange("(j p) d -> p j d", p=128),
                    )
                gt[s] = gp.tile([128, L], F32, tag="g", name=f"gt_{s}")
                nc.sync.dma_start(gt[s][:], gseg[:, offs[s] : offs[s] + L])

            emit_dmas(0)
            nc.sync.dma_start(b1_sb[:], b1s[:])
            nc.sync.dma_start(b2_sb[:], b2s[:])



            for s in range(S):
                L = Ls[s]
                if s + 1 < S:
                    emit_dmas(s + 1)
                # A pass: w1 + gelu; for s>0 interleave w2 (mo 0-1) one m
                # behind gelu. Slot 0 keeps the A pass w1-only so the PE
                # never waits on the still-streaming w2 weights.
                interleave = s > 0
                hid = [None] * HK
                pend = None
                outA = outAps.tile(
                    [128, 2, 512], F32, tag="outA", name=f"outA_{s}"
                )
                for m in range(HK):
                    hps = hidps.tile([128, 512], F32, tag="hps")
                    for k in range(DK):
                        nc.tensor.matmul(
                            hps[:, :L],
                            w1t[s][:, k, m * 128 : (m + 1) * 128],
                            ht[s][:, k, :],
                            start=(k == 0),
                            stop=(k == DK - 1),
                        )
                    hid[m] = hidp.tile(
                        [128, L], F16, tag="hid", name=f"hid_{s}_{m}"
                    )
                    nc.scalar.activation(
                        hid[m][:], hps[:, :L], AF.Gelu,
                        bias=b1_sb[:, s * HK + m : s * HK + m + 1],
                    )
                    if interleave:
                        if pend is not None:
                            for mo in range(2):
                                nc.tensor.matmul(
                                    outA[:, mo, :L],
                                    w2t[s][:, pend, mo * 128 : (mo + 1) * 128],
                                    hid[pend][:],
                                    start=(pend == 0),
                                    stop=False,
                                )
                        pend = m
                if interleave:
                    for mo in range(2):
                        nc.tensor.matmul(
                            outA[:, mo, :L],
                            w2t[s][:, pend, mo * 128 : (mo + 1) * 128],
                            hid[pend][:],
                            start=False,
                            stop=True,
                        )
                else:
                    for m in range(HK):
                        for mo in range(2):
                            nc.tensor.matmul(
                                outA[:, mo, :L],
                                w2t[s][:, m, mo * 128 : (mo + 1) * 128],
                                hid[m][:],
                                start=(m == 0),
                                stop=(m == HK - 1),
                            )
                # B pass: w2 for mo 2-3 (all hid ready; no stalls).
                outB = outBps.tile(
                    [128, 2, 512], F32, tag="outB", name=f"outB_{s}"
                )
                for m in range(HK):
                    for mo in range(2):
                        nc.tensor.matmul(
                            outB[:, mo, :L],
                            w2t[s][:, m, (mo + 2) * 128 : (mo + 3) * 128],
                            hid[m][:],
                            start=(m == 0),
                            stop=(m == HK - 1),
                        )
                # Drain: (psum + b2) * gates -> fp16, single out DMA per slot.
                ot = op.tile([128, DK, L], F16, tag="o", name=f"ot_{s}")
                for mo in range(DK):
                    srcp = outA if mo < 2 else outB
                    nc.vector.scalar_tensor_tensor(
                        ot[:, mo, :],
                        srcp[:, mo % 2, :L],
                        b2_sb[:, s * DK + mo : s * DK + mo + 1],
                        gt[s][:],
                        ALU.add,
                        ALU.mult,
                    )
                nc.sync.dma_start(
                    oseg.rearrange("(m p) n -> p m n", p=128)[
                        :, :, offs[s] : offs[s] + L
                    ],
                    ot[:],
                )
    nc.compile()
    _NC_CACHE[key] = nc
    return nc


def _run(nc, in_maps, label):
    trace = os.environ.get("KTRACE") == "1"
    res = run_bass_kernel_spmd(
        nc, in_maps, core_ids=list(range(NC)), trace=trace
    )
    if trace:
        last_stats[label] = {
            "exec_time_ns": res.exec_time_ns,
            "mean_exec_time_ns": res.mean_exec_time_ns,
            "trace": res.instructions_and_trace[1]
            if res.instructions_and_trace
            else None,
        }
    return res.results


def _gelu_np(x):
    # exact (erf-based) gelu, float32
    try:
        from scipy.special import erf
    except ImportError:
        erf = np.vectorize(math.erf, otypes=[np.float32])
    return 0.5 * x * (1.0 + erf(x / np.sqrt(2.0)))


def kernel(view0, view1, proj_w, proj_b, router_w, expert_keys, w1, b1, w2, b2):
    view0 = np.ascontiguousarray(view0, dtype=np.float32)
    view1 = np.ascontiguousarray(view1, dtype=np.float32)
    proj_w = np.asarray(proj_w, dtype=np.float32)
    proj_b = np.asarray(proj_b, dtype=np.float32)
    router_w = np.asarray(router_w, dtype=np.float32)
    keys = np.asarray(expert_keys, dtype=np.float32)
    w1 = np.asarray(w1, dtype=np.float32)
    b1 = np.asarray(b1, dtype=np.float32)
    w2 = np.asarray(w2, dtype=np.float32)
    b2 = np.asarray(b2, dtype=np.float32)

    # ---- Phase 1: h and d2 on device (token-parallel over 8 cores) ----
    xT_full = np.concatenate(
        [view0.reshape(N, D).T, view1.reshape(N, D).T], axis=1
    )  # [D, NT], column t = view*N + (b*T + tt)
    xT_d = np.ascontiguousarray(xT_full, dtype=np.float16)

    kT2 = np.ascontiguousarray(-2.0 * keys.T).astype(np.float16)  # [D, E]
    kk1 = (keys * keys).sum(axis=1, dtype=np.float32).reshape(1, E)
    onc = np.ones((128, 1), np.float16)
    konr = np.concatenate(
        [np.ones((1, 512), np.float32), kk1], axis=1
    )  # [1, 512+E]

    wc_v = []
    pbc_v = []
    for v in range(V):
        prw = proj_w[v] @ router_w[v]               # [D, D] fp32
        wc = np.concatenate([prw, proj_w[v]], axis=1)  # [D, 2D] = [prw | pw]
        wc_v.append(np.ascontiguousarray(wc, dtype=np.float16))
        prb = proj_b[v] @ router_w[v]               # [D]
        pbc = np.concatenate(
            [prb.reshape(DK, 128).T, proj_b[v].reshape(DK, 128).T], axis=1
        )  # [128, 2*DK] = [prb | pb]
        pbc_v.append(np.ascontiguousarray(pbc, dtype=np.float32))

    in_maps1 = []
    for c in range(NC):
        v = (c * PC) // N  # cores 0-3 -> view 0, 4-7 -> view 1
        in_maps1.append(
            {
                "xT": np.ascontiguousarray(xT_d[:, c * PC : (c + 1) * PC]),
                "wc": wc_v[v],
                "pbc": pbc_v[v],
                "kT2": kT2,
                "konr": konr,
                "onc": onc,
            }
        )
    res1 = _run(_phase1_nc(), in_maps1, "phase1")

    hT_full = np.concatenate([r["hT"] for r in res1], axis=1)  # [D, NT] fp16
    d2 = np.concatenate([r["d2T"] for r in res1], axis=1).T   # [NT, E] fp32

    # ---- Host repair: recompute borderline tokens exactly in fp32 ----
    logits0 = -np.sqrt(np.maximum(d2, 0.0), dtype=np.float32)
    part = np.partition(logits0, E - K - 1, axis=1)
    gap45 = part[:, E - K] - part[:, E - K - 1]  # 4th minus 5th logit
    risk = np.nonzero(gap45 < REPAIR_MARGIN)[0]
    last_stats["n_repaired"] = int(risk.size)
    x_all = np.concatenate([view0.reshape(N, D), view1.reshape(N, D)], axis=0)
    if risk.size:
        vsel = (risk >= N).astype(np.int64)
        kkr = kk1.reshape(E)
        for v in (0, 1):
            rt = risk[vsel == v]
            if rt.size == 0:
                continue
            hx = x_all[rt] @ proj_w[v] + proj_b[v]
            rx = hx @ router_w[v]
            d2[rt] = (
                (rx * rx).sum(axis=1, keepdims=True)
                - 2.0 * (rx @ keys.T)
                + kkr
            )

    # ---- Host routing: logits, top-4, softmax gates (fp32) ----
    logits = -np.sqrt(np.maximum(d2, 0.0), dtype=np.float32)
    topi = np.argsort(-logits, axis=1, kind="stable")[:, :K]   # [NT, K]
    topv = np.take_along_axis(logits, topi, axis=1)
    ex = np.exp(topv - topv[:, :1], dtype=np.float32)
    gates = ex / ex.sum(axis=1, keepdims=True, dtype=np.float32)

    # ---- Partition experts: tiny ones on host, rest on device ----
    fusedT = np.zeros((D, NT), np.float32)
    pieces = []  # (expert, token_ids, gate_vals), each <= LMAX long
    n_host_tok = 0
    for e in range(E):
        sel_tok, sel_k = np.nonzero(topi == e)
        ce = sel_tok.size
        if ce == 0:
            continue
        g_e = gates[sel_tok, sel_k]
        if ce < HOST_EXPERT_MAX:
            # host FFN (exact fp32 from x)
            n_host_tok += ce
            v_of = (sel_tok >= N).astype(np.int64)
            for v in (0, 1):
                msk = v_of == v
                if not msk.any():
                    continue
                tk = sel_tok[msk]
                hx = x_all[tk] @ proj_w[v] + proj_b[v]
                hid = _gelu_np(hx @ w1[e] + b1[e])
                out = hid @ w2[e] + b2[e]
                fusedT[:, tk] += (g_e[msk][:, None] * out).T
            continue
        pieces.append((e, sel_tok, g_e))
    last_stats["n_host_tok"] = n_host_tok

    if not pieces:
        fused = (fusedT[:, :N] + fusedT[:, N:]).T
        return np.ascontiguousarray(fused.reshape(B, T, D), dtype=np.float32)

    # ---- pack pieces into slots ----
    # Per-core load balance is irrelevant: every core executes the full
    # slot-length profile (padded columns have zero gates). So the packing
    # problem is only: pick per-expert piece counts (equalized sizes
    # <= 512), sort all pieces descending, chunk by 8 -> slot length =
    # chunk max. Small search over piece counts minimizes modeled PE time
    # (128 weight tiles per slot put a ~150ns floor per m-tile). Cores are
    # then aligned so consecutive same-expert slots reuse weights (skip
    # the 4MB weight DMA, uniformly across cores).
    import itertools

    empty = (-1, np.zeros(0, np.int64), np.zeros(0, np.float32))
    exps = sorted(pieces, key=lambda p: -p[1].size)

    def profile_for(ns):
        # pieces grouped per expert (experts ordered by piece size desc) so
        # that giant experts fill whole chunks -> weight reuse across their
        # consecutive slots
        groups = []
        for (e, toks, gv), n in zip(exps, ns):
            c = toks.size
            q, r = divmod(c, n)
            groups.append([q + 1] * r + [q] * (n - r))
        groups.sort(key=lambda g: (-g[0], -sum(g)))
        szs = [s for g in groups for s in g]
        prof = [max(szs[i : i + 8]) for i in range(0, len(szs), 8)]
        cost = sum(128 * max(L * 0.4167, 150.0) for L in prof) + 800.0 * len(prof)
        return cost, prof

    base = [math.ceil(p[1].size / LMAX) for p in exps]
    best_ns, best_cost = None, None
    dmax = 4 if len(exps) <= 8 else 2
    for deltas in itertools.product(range(dmax), repeat=len(exps)):
        ns = [b + d for b, d in zip(base, deltas)]
        cost, _ = profile_for(ns)
        if best_cost is None or cost < best_cost:
            best_cost, best_ns = cost, ns

    # rebuild pieces with token ranges for the winning counts, grouped per
    # expert in the same order as profile_for
    piece_groups = []
    for (e, toks, gv), n in zip(exps, best_ns):
        c = toks.size
        bnd = np.linspace(0, c, n + 1).astype(np.int64)
        g = [
            (e, toks[bnd[i] : bnd[i + 1]], gv[bnd[i] : bnd[i + 1]])
            for i in range(n)
        ]
        g.sort(key=lambda p: -p[1].size)
        piece_groups.append(g)
    piece_groups.sort(key=lambda g: (-g[0][1].size, -sum(p[1].size for p in g)))
    all_pieces = [p for g in piece_groups for p in g]
    S = math.ceil(len(all_pieces) / NC)
    chunks = [all_pieces[i * NC : (i + 1) * NC] for i in range(S)]
    for ch in chunks:
        while len(ch) < NC:
            ch.append(empty)
    Ls = tuple(max(p[1].size for p in ch) for ch in chunks)

    # core alignment for weight reuse: same expert on the same core in
    # consecutive slots whenever possible
    core_pieces = [[] for _ in range(NC)]
    prev_e = [None] * NC
    for ch in chunks:
        pool_ch = list(ch)
        slot_assign = [None] * NC
        for c in range(NC):
            for i, p in enumerate(pool_ch):
                if p[0] >= 0 and p[0] == prev_e[c]:
                    slot_assign[c] = pool_ch.pop(i)
                    break
        for c in range(NC):
            if slot_assign[c] is None:
                slot_assign[c] = pool_ch.pop(0)
        for c in range(NC):
            core_pieces[c].append(slot_assign[c])
            prev_e[c] = slot_assign[c][0]
    reuse = tuple(
        s > 0
        and all(
            core_pieces[c][s][0] >= 0
            and core_pieces[c][s][0] == core_pieces[c][s - 1][0]
            for c in range(NC)
        )
        for s in range(S)
    )
    offs = np.concatenate([[0], np.cumsum(Ls)]).astype(np.int64)
    C = int(offs[-1])
    last_stats["S"] = S
    last_stats["Ls"] = Ls
    last_stats["cols_per_core"] = C

    # ---- Phase 2 inputs ----
    w1_d = w1.astype(np.float16)
    w2_d = w2.astype(np.float16)
    in_maps2 = []
    for c in range(NC):
        hsegc = np.zeros((D, C), np.float16)
        gsegc = np.zeros((1, C), np.float32)
        w1c = np.zeros((S, D, H), np.float16)
        w2c = np.zeros((S, H, D), np.float16)
        b1c = np.zeros((128, S * HK), np.float32)
        b2c = np.zeros((128, S * DK), np.float32)
        for s, (e, toks, gv) in enumerate(core_pieces[c]):
            if e < 0:
                continue
            n = toks.size
            o = offs[s]
            hsegc[:, o : o + n] = hT_full[:, toks]
            gsegc[0, o : o + n] = gv
            w1c[s] = w1_d[e]
            w2c[s] = w2_d[e]
            b1c[:, s * HK : (s + 1) * HK] = b1[e].reshape(HK, 128).T
            b2c[:, s * DK : (s + 1) * DK] = b2[e].reshape(DK, 128).T
        in_maps2.append(
            {
                "hseg": hsegc,
                "gseg": np.ascontiguousarray(np.broadcast_to(gsegc, (128, C))),
                "w1s": w1c,
                "w2s": w2c,
                "b1s": b1c,
                "b2s": b2c,
            }
        )
    res2 = _run(_phase2_nc(Ls, reuse), in_maps2, "phase2")
    if os.environ.get("KDEBUG") == "1":
        last_stats["in_maps2"] = in_maps2
        last_stats["res2"] = res2
        last_stats["core_pieces"] = core_pieces
        last_stats["offs"] = offs

    # ---- Combine ----
    for c in range(NC):
        o = res2[c]["oseg"]  # [D, C] fp16
        for s, (e, toks, _gv) in enumerate(core_pieces[c]):
            if e < 0 or toks.size == 0:
                continue
            fusedT[:, toks] += o[:, offs[s] : offs[s] + toks.size].astype(
                np.float32
            )
    fused = (fusedT[:, :N] + fusedT[:, N:]).T  # [N, D]
    return np.ascontiguousarray(fused.reshape(B, T, D), dtype=np.float32)


# revision 31
# speedup vs baseline: 1.0357x; 1.0357x over previous
"""Trainium2 Bass kernel for nn_MoEElementFusion (moe_routing).

Strategy (8 NeuronCores, SPMD, two launches with host routing in between):
  Phase 1 (token-data-parallel): each core takes 1/8 of the 8192 (view,token)
  columns. Host precomputes prw = pw @ rw so h and r are both direct
  projections of x (no serial h->r dependency):
      [r | h] = x @ [prw | pw] + [pb@rw | pb]     (fp16 on the PE, psum fp32)
      d2 = |r|^2 - 2 r.keys^T + |keys|^2
  r tiles come first so the d2 chain (DVE squares -> ones-matmul row sum ->
  keys matmul) overlaps the h matmuls; on the last chunk d2 runs BEFORE the
  h tiles so the kernel tail is just the final h store. Input DMAs are
  split across both HWDGE queues (sync + scalar) in first-use order.

  Host: tokens whose 4th/5th logit gap is under REPAIR_MARGIN get their d2
  row recomputed exactly in fp32; logits = -sqrt(max(d2,0)), stable top-4,
  softmax gates in fp32. Experts with < HOST_EXPERT_MAX selected tokens are
  computed on host (128 weight tiles per slot put a ~19us LDWEIGHTS floor
  on any slot, which tiny experts cannot amortize). The rest are packed:

  Packing insight: every core executes the same slot-length profile
  (padding columns carry zero gates), so per-core balance is irrelevant -
  only the profile sum matters. Each expert is cut into n_e equalized
  expert-pure pieces (<= 512 cols, psum-bank limit); a small search over
  n_e minimizes sum_s max(chunk_s) + per-slot overhead, where chunks are
  consecutive groups of 8 pieces ordered expert-major. Giant experts fill
  whole chunks, and cores are aligned so consecutive same-expert slots
  REUSE the loaded weights (skip the 4MB weight DMA uniformly across
  cores, which kills the startup DMA crunch).

  Phase 2 (compiled at runtime once the profile is known): per slot s of
  length L_s, FFN in fp16 (1 cycle/row on the PE):
      out^T = (w2^T-mm(gelu(w1^T-mm(h^T) + b1)) + b2) * gates
  w2 accumulation is split into two psum groups (mo 0-1 interleaved one
  m-step behind gelu to hide ACT latency, mo 2-3 as a deferred second pass)
  so psum drains always overlap matmuls; slot 0 keeps its A pass w1-only so
  the PE never waits on the still-streaming w2. Weights triple-buffered;
  DMAs are emitted one slot ahead, w1+gates on the sync queue and h+w2 on
  the scalar queue, with output stores last so they never head-of-line
  block the next slot's input streams. psum tiles are allocated at a fixed
  512-column stride (bank alignment) and sliced to L_s.

  Host combine: fused[:, tok] += out columns per slot; sum the two views.
"""

import math
import os

import numpy as np

import concourse.bass as bass
import concourse.bacc as bacc
import concourse.mybir as mybir
import concourse.tile as tile
from concourse.bass_utils import run_bass_kernel_spmd

# Problem dims (hardcoded per spec)
V, B, T, D, E, K = 2, 4, 1024, 512, 16, 4
H = 4 * D
N = B * T          # tokens per view
NT = V * N         # total (view, token) columns = 8192
NC = 8             # cores
PC = NT // NC      # phase-1 columns per core = 1024
LMAX = 512         # max phase-2 slot length (psum bank limit)

F32 = mybir.dt.float32
F16 = mybir.dt.float16
AF = mybir.ActivationFunctionType
ALU = mybir.AluOpType

DK = D // 128      # 4 k-tiles over D
HK = H // 128      # 16 k-tiles over H

REPAIR_MARGIN = 0.02
HOST_EXPERT_MAX = 256   # experts with fewer selected tokens run on host

# Filled by kernel() for test harness introspection.
last_stats: dict = {}

_NC_CACHE: dict = {}


def _phase1_nc() -> bass.Bass:
    if "p1" in _NC_CACHE:
        return _NC_CACHE["p1"]
    nc = bacc.Bacc("TRN2", target_bir_lowering=False, num_devices=NC)
    xT = nc.dram_tensor("xT", [D, PC], F16, kind="ExternalInput")
    # wc = [prw | pw]: router projection first (host wants r early for d2)
    wc = nc.dram_tensor("wc", [D, 2 * D], F16, kind="ExternalInput")
    pbc = nc.dram_tensor("pbc", [128, 2 * DK], F32, kind="ExternalInput")
    hT = nc.dram_tensor("hT", [D, PC], F16, kind="ExternalOutput")
    rT = nc.dram_tensor("rT", [D, PC], F16, kind="ExternalOutput")

    NCH = PC // 512  # 512-column chunks

    with tile.TileContext(nc) as tc:
        with (
            tc.tile_pool(name="const", bufs=1) as cpool,
            tc.tile_pool(name="act", bufs=1) as apool,
            tc.tile_pool(name="ps", bufs=6, space="PSUM") as pspool,
        ):
            xT_sb = cpool.tile([128, DK, PC], F16, tag="xT")
            wc_sb = cpool.tile([128, DK, 2 * D], F16, tag="wc")
            # Feed order = first-use order, split across both HWDGE queues:
            # prw slabs + first xT chunk fine-grained, the rest merged.
            for k in range(DK):
                q = nc.sync if k % 2 == 0 else nc.scalar
                q.dma_start(
                    wc_sb[:, k, 0:D], wc[k * 128 : (k + 1) * 128, 0:D]
                )
                q.dma_start(
                    xT_sb[:, k, 0:512], xT[k * 128 : (k + 1) * 128, 0:512]
                )
            nc.sync.dma_start(
                wc_sb[:, :, D : 2 * D],
                wc.rearrange("(k p) d -> p k d", p=128)[:, :, D : 2 * D],
            )
            nc.scalar.dma_start(
                xT_sb[:, :, 512:PC],
                xT.rearrange("(k p) n -> p k n", p=128)[:, :, 512:PC],
            )
            pbc_sb = cpool.tile([128, 2 * DK], F32, tag="pbc")
            nc.scalar.dma_start(pbc_sb[:], pbc[:])

            hT_sb = apool.tile([128, DK, PC], F16, tag="hT")
            rT_sb = apool.tile([128, DK, PC], F16, tag="rT")

            # [r | h]^T tiles = wc^T-mm(x^T) + pbc; r first, streamed out per
            # half-chunk so the host can start d2/top-k as early as possible.
            for n in range(NCH):
                cols = slice(n * 512, (n + 1) * 512)
                for m in range(2 * DK):
                    ps = pspool.tile([128, 512], F32, tag="ps")
                    for k in range(DK):
                        nc.tensor.matmul(
                            ps[:],
                            wc_sb[:, k, m * 128 : (m + 1) * 128],
                            xT_sb[:, k, cols],
                            start=(k == 0),
                            stop=(k == DK - 1),
                        )
                    dst = rT_sb if m < DK else hT_sb
                    mm = m if m < DK else m - DK
                    nc.scalar.activation(
                        dst[:, mm, cols], ps[:],
                        AF.Identity, bias=pbc_sb[:, m : m + 1],
                    )
                    if m == DK - 1:
                        nc.scalar.dma_start(
                            rT.rearrange("(m p) n -> p m n", p=128)[:, :, cols],
                            rT_sb[:, :, cols],
                        )
                nc.scalar.dma_start(
                    hT.rearrange("(m p) n -> p m n", p=128)[:, :, cols],
                    hT_sb[:, :, cols],
                )
    nc.compile()
    _NC_CACHE["p1"] = nc
    return nc


def _phase2_nc(Ls: tuple, reuse: tuple) -> bass.Bass:
    """FFN over S slots with compile-time lengths Ls (each <= 512)."""
    key = ("p2", Ls, reuse)
    if key in _NC_CACHE:
        return _NC_CACHE[key]
    nc = bacc.Bacc("TRN2", target_bir_lowering=False, num_devices=NC)
    S = len(Ls)
    offs = [0]
    for L in Ls:
        offs.append(offs[-1] + L)
    C = offs[-1]
    hseg = nc.dram_tensor("hseg", [D, C], F16, kind="ExternalInput")
    gseg = nc.dram_tensor("gseg", [128, C], F32, kind="ExternalInput")
    w1s = nc.dram_tensor("w1s", [S, D, H], F16, kind="ExternalInput")
    w2s = nc.dram_tensor("w2s", [S, H, D], F16, kind="ExternalInput")
    b1s = nc.dram_tensor("b1s", [128, S * HK], F32, kind="ExternalInput")
    b2s = nc.dram_tensor("b2s", [128, S * DK], F32, kind="ExternalInput")
    oseg = nc.dram_tensor("oseg", [D, C], F16, kind="ExternalOutput")

    with tile.TileContext(nc) as tc:
        with (
            tc.tile_pool(name="const", bufs=1) as cpool,
            tc.tile_pool(name="w1p", bufs=3) as w1p,
            tc.tile_pool(name="w2p", bufs=3) as w2p,
            tc.tile_pool(name="hp", bufs=3) as hp,
            tc.tile_pool(name="gp", bufs=3) as gp,
            tc.tile_pool(name="hidp", bufs=HK + 2) as hidp,
            tc.tile_pool(name="op", bufs=2) as op,
            tc.tile_pool(name="hid_ps", bufs=2, space="PSUM") as hidps,
            tc.tile_pool(name="outA_ps", bufs=1, space="PSUM") as outAps,
            tc.tile_pool(name="outB_ps", bufs=1, space="PSUM") as outBps,
        ):
            b1_sb = cpool.tile([128, S * HK], F32, tag="b1")
            b2_sb = cpool.tile([128, S * DK], F32, tag="b2")

            w1t = [None] * S
            w2t = [None] * S
            ht = [None] * S
            gt = [None] * S

            def emit_dmas(s):
                # w1 + gates on the sync HWDGE queue; h and w2 on the scalar
                # HWDGE queue (h first: A-pass needs it, w2 only at B-pass).
                # Slot 0 uses per-k descriptors so the first m-tiles arrive
                # early; prefetched slots use one descriptor per tensor to
                # cut issue time and semaphore teardown. Band slots with
                # reuse[s] keep the previous slot's weight tiles (no DMA).
                L = Ls[s]
                ht[s] = hp.tile([128, DK, L], F16, tag="h", name=f"ht_{s}")
                if s == 0:
                    for k in range(DK):
                        nc.scalar.dma_start(
                            ht[s][:, k, :],
                            hseg[k * 128 : (k + 1) * 128, offs[s] : offs[s] + L],
                        )
                else:
                    nc.scalar.dma_start(
                        ht[s][:],
                        hseg.rearrange("(k p) n -> p k n", p=128)[
                            :, :, offs[s] : offs[s] + L
                        ],
                    )
                if reuse[s]:
                    w1t[s] = w1t[s - 1]
                    w2t[s] = w2t[s - 1]
                else:
                    w1t[s] = w1p.tile(
                        [128, DK, H], F16, tag="w1", name=f"w1t_{s}"
                    )
                    if s == 0:
                        for k in range(DK):
                            for hh in range(2):
                                nc.sync.dma_start(
                                    w1t[s][
                                        :, k,
                                        hh * (H // 2) : (hh + 1) * (H // 2),
                                    ],
                                    w1s[
                                        s, k * 128 : (k + 1) * 128,
                                        hh * (H // 2) : (hh + 1) * (H // 2),
                                    ],
                                )
                    else:
                        nc.sync.dma_start(
                            w1t[s][:],
                            w1s[s].rearrange("(k p) h -> p k h", p=128),
                        )
                    w2t[s] = w2p.tile(
                        [128, HK, D], F16, tag="w2", name=f"w2t_{s}"
                    )
                    nc.scalar.dma_start(
                        w2t[s][:],
                        w2s[s].rearrange("(j p) d -> p j d", p=128),
                    )
                gt[s] = gp.tile([128, L], F32, tag="g", name=f"gt_{s}")
                nc.sync.dma_start(gt[s][:], gseg[:, offs[s] : offs[s] + L])

            emit_dmas(0)
            nc.sync.dma_start(b1_sb[:], b1s[:])
            nc.sync.dma_start(b2_sb[:], b2s[:])



            for s in range(S):
                L = Ls[s]
                if s + 1 < S:
                    emit_dmas(s + 1)
                # A pass: w1 + gelu; for s>0 interleave w2 (mo 0-1) one m
                # behind gelu. Slot 0 keeps the A pass w1-only so the PE
                # never waits on the still-streaming w2 weights.
                interleave = s > 0
                hid = [None] * HK
                pend = None
                outA = outAps.tile(
                    [128, 2, 512], F32, tag="outA", name=f"outA_{s}"
                )
                for m in range(HK):
                    hps = hidps.tile([128, 512], F32, tag="hps")
                    for k in range(DK):
                        nc.tensor.matmul(
                            hps[:, :L],
                            w1t[s][:, k, m * 128 : (m + 1) * 128],
                            ht[s][:, k, :],
                            start=(k == 0),
                            stop=(k == DK - 1),
                        )
                    hid[m] = hidp.tile(
                        [128, L], F16, tag="hid", name=f"hid_{s}_{m}"
                    )
                    nc.scalar.activation(
                        hid[m][:], hps[:, :L], AF.Gelu,
                        bias=b1_sb[:, s * HK + m : s * HK + m + 1],
                    )
                    if interleave:
                        if pend is not None:
                            for mo in range(2):
                                nc.tensor.matmul(
                                    outA[:, mo, :L],
                                    w2t[s][:, pend, mo * 128 : (mo + 1) * 128],
                                    hid[pend][:],
                                    start=(pend == 0),
                                    stop=False,
                                )
                        pend = m
                if interleave:
                    for mo in range(2):
                        nc.tensor.matmul(
                            outA[:, mo, :L],
                            w2t[s][:, pend, mo * 128 : (mo + 1) * 128],
                            hid[pend][:],
                            start=False,
                            stop=True,
                        )
                else:
                    for m in range(HK):
                        for mo in range(2):
                            nc.tensor.matmul(
                                outA[:, mo, :L],
                                w2t[s][:, m, mo * 128 : (mo + 1) * 128],
                                hid[m][:],
                                start=(m == 0),
                                stop=(m == HK - 1),
                            )
                # B pass: w2 for mo 2-3 (all hid ready; no stalls).
                outB = outBps.tile(
                    [128, 2, 512], F32, tag="outB", name=f"outB_{s}"
                )
                for m in range(HK):
                    for mo in range(2):
                        nc.tensor.matmul(
                            outB[:, mo, :L],
                            w2t[s][:, m, (mo + 2) * 128 : (mo + 3) * 128],
                            hid[m][:],
                            start=(m == 0),
                            stop=(m == HK - 1),
                        )
                # Drain: (psum + b2) * gates -> fp16, single out DMA per slot.
                ot = op.tile([128, DK, L], F16, tag="o", name=f"ot_{s}")
                for mo in range(DK):
                    srcp = outA if mo < 2 else outB
                    nc.vector.scalar_tensor_tensor(
                        ot[:, mo, :],
                        srcp[:, mo % 2, :L],
                        b2_sb[:, s * DK + mo : s * DK + mo + 1],
                        gt[s][:],
                        ALU.add,
                        ALU.mult,
                    )
                nc.sync.dma_start(
                    oseg.rearrange("(m p) n -> p m n", p=128)[
                        :, :, offs[s] : offs[s] + L
                    ],
                    ot[:],
                )
    nc.compile()
    _NC_CACHE[key] = nc
    return nc


def _run(nc, in_maps, label):
    trace = os.environ.get("KTRACE") == "1"
    res = run_bass_kernel_spmd(
        nc, in_maps, core_ids=list(range(NC)), trace=trace
    )
    if trace:
        last_stats[label] = {
            "exec_time_ns": res.exec_time_ns,
            "mean_exec_time_ns": res.mean_exec_time_ns,
            "trace": res.instructions_and_trace[1]
            if res.instructions_and_trace
            else None,
        }
    return res.results


def _gelu_np(x):
    # exact (erf-based) gelu, float32
    try:
        from scipy.special import erf
    except ImportError:
        erf = np.vectorize(math.erf, otypes=[np.float32])
    return 0.5 * x * (1.0 + erf(x / np.sqrt(2.0)))


def kernel(view0, view1, proj_w, proj_b, router_w, expert_keys, w1, b1, w2, b2):
    view0 = np.ascontiguousarray(view0, dtype=np.float32)
    view1 = np.ascontiguousarray(view1, dtype=np.float32)
    proj_w = np.asarray(proj_w, dtype=np.float32)
    proj_b = np.asarray(proj_b, dtype=np.float32)
    router_w = np.asarray(router_w, dtype=np.float32)
    keys = np.asarray(expert_keys, dtype=np.float32)
    w1 = np.asarray(w1, dtype=np.float32)
    b1 = np.asarray(b1, dtype=np.float32)
    w2 = np.asarray(w2, dtype=np.float32)
    b2 = np.asarray(b2, dtype=np.float32)

    # ---- Phase 1: h and d2 on device (token-parallel over 8 cores) ----
    xT_full = np.concatenate(
        [view0.reshape(N, D).T, view1.reshape(N, D).T], axis=1
    )  # [D, NT], column t = view*N + (b*T + tt)
    xT_d = np.ascontiguousarray(xT_full, dtype=np.float16)

    kk1 = (keys * keys).sum(axis=1, dtype=np.float32).reshape(1, E)

    wc_v = []
    pbc_v = []
    for v in range(V):
        prw = proj_w[v] @ router_w[v]               # [D, D] fp32
        wc = np.concatenate([prw, proj_w[v]], axis=1)  # [D, 2D] = [prw | pw]
        wc_v.append(np.ascontiguousarray(wc, dtype=np.float16))
        prb = proj_b[v] @ router_w[v]               # [D]
        pbc = np.concatenate(
            [prb.reshape(DK, 128).T, proj_b[v].reshape(DK, 128).T], axis=1
        )  # [128, 2*DK] = [prb | pb]
        pbc_v.append(np.ascontiguousarray(pbc, dtype=np.float32))

    in_maps1 = []
    for c in range(NC):
        v = (c * PC) // N  # cores 0-3 -> view 0, 4-7 -> view 1
        in_maps1.append(
            {
                "xT": np.ascontiguousarray(xT_d[:, c * PC : (c + 1) * PC]),
                "wc": wc_v[v],
                "pbc": pbc_v[v],
            }
        )
    res1 = _run(_phase1_nc(), in_maps1, "phase1")

    hT_full = np.concatenate([r["hT"] for r in res1], axis=1)  # [D, NT] fp16
    r_full = np.concatenate(
        [r["rT"] for r in res1], axis=1
    ).T.astype(np.float32)                                    # [NT, D]
    d2 = (
        (r_full * r_full).sum(axis=1, keepdims=True)
        - 2.0 * (r_full @ keys.T)
        + kk1
    )                                                          # [NT, E] fp32

    # ---- Host repair: recompute borderline tokens exactly in fp32 ----
    logits0 = -np.sqrt(np.maximum(d2, 0.0), dtype=np.float32)
    part = np.partition(logits0, E - K - 1, axis=1)
    gap45 = part[:, E - K] - part[:, E - K - 1]  # 4th minus 5th logit
    risk = np.nonzero(gap45 < REPAIR_MARGIN)[0]
    last_stats["n_repaired"] = int(risk.size)
    x_all = np.concatenate([view0.reshape(N, D), view1.reshape(N, D)], axis=0)
    if risk.size:
        vsel = (risk >= N).astype(np.int64)
        kkr = kk1.reshape(E)
        for v in (0, 1):
            rt = risk[vsel == v]
            if rt.size == 0:
                continue
            hx = x_all[rt] @ proj_w[v] + proj_b[v]
            rx = hx @ router_w[v]
            d2[rt] = (
                (rx * rx).sum(axis=1, keepdims=True)
                - 2.0 * (rx @ keys.T)
                + kkr
            )

    # ---- Host routing: logits, top-4, softmax gates (fp32) ----
    logits = -np.sqrt(np.maximum(d2, 0.0), dtype=np.float32)
    topi = np.argsort(-logits, axis=1, kind="stable")[:, :K]   # [NT, K]
    topv = np.take_along_axis(logits, topi, axis=1)
    ex = np.exp(topv - topv[:, :1], dtype=np.float32)
    gates = ex / ex.sum(axis=1, keepdims=True, dtype=np.float32)

    # ---- Partition experts: tiny ones on host, rest on device ----
    fusedT = np.zeros((D, NT), np.float32)
    pieces = []  # (expert, token_ids, gate_vals), each <= LMAX long
    n_host_tok = 0
    for e in range(E):
        sel_tok, sel_k = np.nonzero(topi == e)
        ce = sel_tok.size
        if ce == 0:
            continue
        g_e = gates[sel_tok, sel_k]
        if ce < HOST_EXPERT_MAX:
            # host FFN (exact fp32 from x)
            n_host_tok += ce
            v_of = (sel_tok >= N).astype(np.int64)
            for v in (0, 1):
                msk = v_of == v
                if not msk.any():
                    continue
                tk = sel_tok[msk]
                hx = x_all[tk] @ proj_w[v] + proj_b[v]
                hid = _gelu_np(hx @ w1[e] + b1[e])
                out = hid @ w2[e] + b2[e]
                fusedT[:, tk] += (g_e[msk][:, None] * out).T
            continue
        pieces.append((e, sel_tok, g_e))
    last_stats["n_host_tok"] = n_host_tok

    if not pieces:
        fused = (fusedT[:, :N] + fusedT[:, N:]).T
        return np.ascontiguousarray(fused.reshape(B, T, D), dtype=np.float32)

    # ---- pack pieces into slots ----
    # Per-core load balance is irrelevant: every core executes the full
    # slot-length profile (padded columns have zero gates). So the packing
    # problem is only: pick per-expert piece counts (equalized sizes
    # <= 512), sort all pieces descending, chunk by 8 -> slot length =
    # chunk max. Small search over piece counts minimizes modeled PE time
    # (128 weight tiles per slot put a ~150ns floor per m-tile). Cores are
    # then aligned so consecutive same-expert slots reuse weights (skip
    # the 4MB weight DMA, uniformly across cores).
    import itertools

    empty = (-1, np.zeros(0, np.int64), np.zeros(0, np.float32))
    exps = sorted(pieces, key=lambda p: -p[1].size)

    def profile_for(ns):
        # pieces grouped per expert (experts ordered by piece size desc) so
        # that giant experts fill whole chunks -> weight reuse across their
        # consecutive slots
        groups = []
        for (e, toks, gv), n in zip(exps, ns):
            c = toks.size
            q, r = divmod(c, n)
            groups.append([q + 1] * r + [q] * (n - r))
        groups.sort(key=lambda g: (-g[0], -sum(g)))
        szs = [s for g in groups for s in g]
        prof = [max(szs[i : i + 8]) for i in range(0, len(szs), 8)]
        cost = sum(128 * max(L * 0.4167, 150.0) for L in prof) + 800.0 * len(prof)
        return cost, prof

    base = [math.ceil(p[1].size / LMAX) for p in exps]
    best_ns, best_cost = None, None
    dmax = 4 if len(exps) <= 8 else 2
    for deltas in itertools.product(range(dmax), repeat=len(exps)):
        ns = [b + d for b, d in zip(base, deltas)]
        cost, _ = profile_for(ns)
        if best_cost is None or cost < best_cost:
            best_cost, best_ns = cost, ns

    # rebuild pieces with token ranges for the winning counts, grouped per
    # expert in the same order as profile_for
    piece_groups = []
    for (e, toks, gv), n in zip(exps, best_ns):
        c = toks.size
        bnd = np.linspace(0, c, n + 1).astype(np.int64)
        g = [
            (e, toks[bnd[i] : bnd[i + 1]], gv[bnd[i] : bnd[i + 1]])
            for i in range(n)
        ]
        g.sort(key=lambda p: -p[1].size)
        piece_groups.append(g)
    piece_groups.sort(key=lambda g: (-g[0][1].size, -sum(p[1].size for p in g)))
    all_pieces = [p for g in piece_groups for p in g]
    S = math.ceil(len(all_pieces) / NC)
    chunks = [all_pieces[i * NC : (i + 1) * NC] for i in range(S)]
    for ch in chunks:
        while len(ch) < NC:
            ch.append(empty)
    Ls = tuple(max(p[1].size for p in ch) for ch in chunks)

    # core alignment for weight reuse: same expert on the same core in
    # consecutive slots whenever possible
    core_pieces = [[] for _ in range(NC)]
    prev_e = [None] * NC
    for ch in chunks:
        pool_ch = list(ch)
        slot_assign = [None] * NC
        for c in range(NC):
            for i, p in enumerate(pool_ch):
                if p[0] >= 0 and p[0] == prev_e[c]:
                    slot_assign[c] = pool_ch.pop(i)
                    break
        for c in range(NC):
            if slot_assign[c] is None:
                slot_assign[c] = pool_ch.pop(0)
        for c in range(NC):
            core_pieces[c].append(slot_assign[c])
            prev_e[c] = slot_assign[c][0]
    reuse = tuple(
        s > 0
        and all(
            core_pieces[c][s][0] >= 0
            and core_pieces[c][s][0] == core_pieces[c][s - 1][0]
            for c in range(NC)
        )
        for s in range(S)
    )
    offs = np.concatenate([[0], np.cumsum(Ls)]).astype(np.int64)
    C = int(offs[-1])
    last_stats["S"] = S
    last_stats["Ls"] = Ls
    last_stats["cols_per_core"] = C

    # ---- Phase 2 inputs ----
    w1_d = w1.astype(np.float16)
    w2_d = w2.astype(np.float16)
    in_maps2 = []
    for c in range(NC):
        hsegc = np.zeros((D, C), np.float16)
        gsegc = np.zeros((1, C), np.float32)
        w1c = np.zeros((S, D, H), np.float16)
        w2c = np.zeros((S, H, D), np.float16)
        b1c = np.zeros((128, S * HK), np.float32)
        b2c = np.zeros((128, S * DK), np.float32)
        for s, (e, toks, gv) in enumerate(core_pieces[c]):
            if e < 0:
                continue
            n = toks.size
            o = offs[s]
            hsegc[:, o : o + n] = hT_full[:, toks]
            gsegc[0, o : o + n] = gv
            w1c[s] = w1_d[e]
            w2c[s] = w2_d[e]
            b1c[:, s * HK : (s + 1) * HK] = b1[e].reshape(HK, 128).T
            b2c[:, s * DK : (s + 1) * DK] = b2[e].reshape(DK, 128).T
        in_maps2.append(
            {
                "hseg": hsegc,
                "gseg": np.ascontiguousarray(np.broadcast_to(gsegc, (128, C))),
                "w1s": w1c,
                "w2s": w2c,
                "b1s": b1c,
                "b2s": b2c,
            }
        )
    res2 = _run(_phase2_nc(Ls, reuse), in_maps2, "phase2")
    if os.environ.get("KDEBUG") == "1":
        last_stats["in_maps2"] = in_maps2
        last_stats["res2"] = res2
        last_stats["core_pieces"] = core_pieces
        last_stats["offs"] = offs

    # ---- Combine ----
    for c in range(NC):
        o = res2[c]["oseg"]  # [D, C] fp16
        for s, (e, toks, _gv) in enumerate(core_pieces[c]):
            if e < 0 or toks.size == 0:
                continue
            fusedT[:, toks] += o[:, offs[s] : offs[s] + toks.size].astype(
                np.float32
            )
    fused = (fusedT[:, :N] + fusedT[:, N:]).T  # [N, D]
    return np.ascontiguousarray(fused.reshape(B, T, D), dtype=np.float32)
